# revision 5
# baseline (speedup 1.0000x reference)
"""Trainium2 Bass kernel for nn_Net_5488968204310 (gnn_message_passing).

Single-head self-attention (D=128) over N=1024 nodes + gated residual update,
batch B=32, data-parallel across 8 NeuronCores (4 samples per core).

Design notes:
  - "T layout" (features d on partitions, nodes on free dim) for every matmul;
    all eight 128x128 weight matmuls contract over d.
  - x is uploaded twice: fp32 (final residual add) and bf16 (DMA-transposed
    straight into xT; layout transposes run on the DMA xbar, not the PE).
  - QK^T: logitsT chunks [m_chunk(128) x q(1024)] = kT_chunk.T @ qT; exp() on
    the scalar engine directly from PSUM with the 1/sqrt(D) scale folded in.
  - AV keeps v as the stationary operand (few LDWEIGHTS, dense 512-col
    streams): attnT[d,q] = sum_c v_nat[c].T @ expw[c]. The softmax denominator
    is a parallel ones.T @ expw accumulation; 1/denom via the DVE
    reciprocal_approx_fast custom op (1 DVE instr, ~18 bits), freeing the
    scalar engine and making the rb write->read ordering explicit per-half.
  - gate sigmoid as 0.5*(1+tanh(z/2)): one ACT op (Tanh) instead of the
    3-op exp/ln chain. The 0.5 gate factor is folded into Wo/Wo1m/bo_u on
    the host so dlt = u_half * (tanh+1) is a single DVE scalar_tensor_tensor.
  - Every ACT function used (Exp, Tanh, Identity/Relu fallbacks) lives in the
    exp_and_others table set, enforced by a scoped patch of the table
    metadata at compile time, so there is exactly one ACT_TABLE_LOAD per run.
  - relu of the gate pre-activation runs on the (otherwise idle) Pool engine.
  - Host folds: Wo1 -> Wo1 - I (so x@(Wo1-I)+msg = ret-x directly),
    Wo@Wg2 (gate path skips msg), bv -> bo terms, bias sums, 0.5 gate factor.
"""

import math

import numpy as np
import ml_dtypes

B, N, D = 32, 1024, 128
NCORES = 8
BPC = B // NCORES  # samples per core
NT = N // 128      # node chunks per sample

_CACHE = {}


def _bias_mode(vec):
    """(kind, value) where kind in {'zero', 'uniform', 'ap'}."""
    v = np.asarray(vec, np.float32)
    if not np.any(v):
        return ("zero", 0.0)
    if np.all(v == v.flat[0]):
        return ("uniform", float(v.flat[0]))
    return ("ap", 0.0)


def _build_nc(modes):
    import concourse.bacc as bacc
    import concourse.tile as tile
    from concourse import mybir
    from contextlib import ExitStack

    f32 = mybir.dt.float32
    bf16 = mybir.dt.bfloat16
    f8 = mybir.dt.float8e4
    AF = mybir.ActivationFunctionType
    OP = mybir.AluOpType

    nc = bacc.Bacc("TRN2", target_bir_lowering=False, debug=False)

    x_d = nc.dram_tensor("x", [BPC, N, D], f32, kind="ExternalInput")
    xb_d = nc.dram_tensor("xbf", [BPC, N, D], bf16, kind="ExternalInput")
    out_d = nc.dram_tensor("out", [BPC, N, D], f32, kind="ExternalOutput")
    wnames = ["Wq", "Wk", "Wv", "Woh", "Wo1mh", "Wg1", "Wog2", "Wg3"]
    w_d = {n: nc.dram_tensor(n, [D, D], bf16, kind="ExternalInput") for n in wnames}
    b_d = {
        n: nc.dram_tensor(n, [D, 1], f32, kind="ExternalInput")
        for n in modes if modes[n][0] == "ap"
    }

    s = 1.0 / math.sqrt(D)

    with tile.TileContext(nc) as tc, ExitStack() as ctx:
        consts = ctx.enter_context(tc.tile_pool(name="consts", bufs=1))
        sb = ctx.enter_context(tc.tile_pool(name="sb", bufs=2))
        sb4 = ctx.enter_context(tc.tile_pool(name="sb4", bufs=4))
        expp = ctx.enter_context(tc.tile_pool(name="expp", bufs=2))
        pw = ctx.enter_context(tc.tile_pool(name="pw", bufs=2, space="PSUM"))
        ph = ctx.enter_context(tc.tile_pool(name="ph", bufs=2, space="PSUM"))
        pav = ctx.enter_context(tc.tile_pool(name="pav", bufs=1, space="PSUM"))
        pden = ctx.enter_context(tc.tile_pool(name="pden", bufs=1, space="PSUM"))

        W = {}
        for n in wnames:
            t = consts.tile([D, D], bf16, tag=f"w_{n}")
            nc.sync.dma_start(t, w_d[n][:, :])
            W[n] = t
        ones_dr = consts.tile([128, 2, 128], f8, tag="ones_dr")
        nc.vector.memset(ones_dr, 1.0)
        expbias = consts.tile([128, 1], f32, tag="expbias")
        nc.vector.memset(expbias, -2.0)
        BV = {}
        for n in b_d:
            t = consts.tile([D, 1], f32, tag=f"b_{n}")
            nc.sync.dma_start(t, b_d[n][:, :])
            BV[n] = t
        for n, (kind, val) in modes.items():
            if kind == "uniform":
                t = consts.tile([D, 1], f32, tag=f"b_{n}")
                nc.vector.memset(t, val)
                BV[n] = t

        def copyback(dst, src, bname, engine_copy):
            """psum->sbuf copy honoring the bias mode for `bname`."""
            kind, val = modes[bname]
            if kind == "zero":
                engine_copy(dst, src)
            else:
                nc.scalar.activation(dst, src, AF.Identity, bias=BV[bname])

        ST = {}

        def load(b):
            """input DMAs for sample b (issued one pipeline step early)."""
            st = {}
            x_nat = sb4.tile([128, NT, D], f32, tag="x_nat")
            nc.sync.dma_start(x_nat, x_d[b].rearrange("(c p) d -> p c d", p=128))
            xT = sb4.tile([128, N], bf16, tag="xT")  # [d, n]
            nc.sync.dma_start_transpose(xT, xb_d[b])
            st["x_nat"], st["xT"] = x_nat, xT
            return st

        def phase1(st):
            """q/k/v projections, QK^T + exp."""
            xT = st["xT"]

            p_q = pw.tile([128, N], f32, tag="pw")
            nc.tensor.matmul(p_q[:, 0:512], W["Wq"], xT[:, 0:512], start=True, stop=True)
            nc.tensor.matmul(p_q[:, 512:1024], W["Wq"], xT[:, 512:1024], start=True, stop=True)
            qT = sb.tile([128, N], bf16, tag="qT")
            copyback(qT, p_q, "bq", nc.vector.tensor_copy)

            p_k = pw.tile([128, N], f32, tag="pw")
            nc.tensor.matmul(p_k[:, 0:512], W["Wk"], xT[:, 0:512], start=True, stop=True)
            nc.tensor.matmul(p_k[:, 512:1024], W["Wk"], xT[:, 512:1024], start=True, stop=True)
            kT = sb.tile([128, N], bf16, tag="kT")
            copyback(kT, p_k, "bk", nc.vector.tensor_copy)

            p_v = pw.tile([128, N], f32, tag="pw")
            for c in range(NT):
                nc.tensor.matmul(p_v[:, c * 128:(c + 1) * 128], xT[:, c * 128:(c + 1) * 128], W["Wv"], start=True, stop=True)
            v_nat = sb.tile([128, NT, 128], f8, tag="v_nat")
            nc.vector.tensor_copy(v_nat, p_v.rearrange("p (c n) -> p c n", c=NT))
            st["v_nat"] = v_nat

            # exp output in fp8e4m3: bias -2 rescales exp into fp8 range; the
            # uniform factor e^-2 cancels between numerator and denominator.
            expw = expp.tile([128, NT, N], f8, tag="expw")  # [m', c_m, q]
            for c in range(NT):
                p_l = pw.tile([128, N], f32, tag="pw")
                kTc = kT[:, c * 128:(c + 1) * 128]
                nc.tensor.matmul(p_l[:, 0:512], kTc, qT[:, 0:512], start=True, stop=True)
                nc.tensor.matmul(p_l[:, 512:1024], kTc, qT[:, 512:1024], start=True, stop=True)
                nc.scalar.activation(expw[:, c, :], p_l, AF.Exp, scale=s, bias=expbias)
            st["expw"] = expw

        def phase2(st):
            """denominator + reciprocal + AV + normalize -> attnT."""
            expw, v_nat = st["expw"], st["v_nat"]
            rb = sb.tile([128, N], f32, tag="rb")
            attnT = sb.tile([128, N], bf16, tag="attnT")
            for h in range(2):
                sl = slice(h * 512, (h + 1) * 512)
                p_dn = pden.tile([128, 512], f32, tag="pden")
                for c in range(NT // 2):
                    nc.tensor.matmul(
                        p_dn, ones_dr, expw[:, 2 * c:2 * c + 2, sl],
                        start=(c == 0), stop=(c == NT // 2 - 1),
                        perf_mode=mybir.MatmulPerfMode.DoubleRow,
                    )
                nc.vector.reciprocal_approx_fast(rb[:, sl], p_dn)
                p_av = pav.tile([128, 512], f32, tag="pav")
                for c in range(NT // 2):
                    nc.tensor.matmul(
                        p_av, v_nat[:, 2 * c:2 * c + 2, :], expw[:, 2 * c:2 * c + 2, sl],
                        start=(c == 0), stop=(c == NT // 2 - 1),
                        perf_mode=mybir.MatmulPerfMode.DoubleRow,
                    )
                nc.vector.tensor_mul(attnT[:, sl], p_av, rb[:, sl])
            st["attnT"] = attnT

        def phase3(b, st):
            """gated update tail in 512-wide halves; store.

            u = 0.5*(ret - x) comes straight out of the halved-weight matmul;
            gate = 0.5*(1 + tanh((z+bg3)/2)); out = x + u*(1+tanh)."""
            x_nat, xT, attnT = st["x_nat"], st["xT"], st["attnT"]
            u = sb.tile([128, N], f32, tag="u")
            gp = sb.tile([128, N], bf16, tag="gp")
            th = sb.tile([128, N], bf16, tag="th")
            dlt = sb.tile([128, N], bf16, tag="dlt")
            dlt_nat = sb.tile([128, NT, 128], bf16, tag="dlt_nat")
            o = sb.tile([128, NT, D], f32, tag="o")
            out_r = out_d[b].rearrange("(c p) d -> p c d", p=128)
            H = NT // 2
            tanh_bias = BV["bg3h"] if "bg3h" in BV else 0.0
            for h in range(2):
                sl = slice(h * 512, (h + 1) * 512)
                cs = slice(h * H, (h + 1) * H)

                p_m = ph.tile([128, 512], f32, tag="pwh")
                nc.tensor.matmul(p_m, W["Woh"], attnT[:, sl], start=True, stop=False)
                nc.tensor.matmul(p_m, W["Wo1mh"], xT[:, sl], start=False, stop=True)
                copyback(u[:, sl], p_m, "bo_uh", nc.vector.tensor_copy)

                p_g = ph.tile([128, 512], f32, tag="pwh")
                nc.tensor.matmul(p_g, W["Wg1"], xT[:, sl], start=True, stop=False)
                nc.tensor.matmul(p_g, W["Wog2"], attnT[:, sl], start=False, stop=True)
                if modes["bo_g"][0] == "zero":
                    nc.vector.tensor_scalar(gp[:, sl], p_g, 0.0, None, op0=OP.max)
                else:
                    nc.scalar.activation(gp[:, sl], p_g, AF.Relu, bias=BV["bo_g"])

                p_g3 = ph.tile([128, 512], f32, tag="pwh")
                nc.tensor.matmul(p_g3, W["Wg3"], gp[:, sl], start=True, stop=True)
                nc.scalar.activation(th[:, sl], p_g3, AF.Tanh, scale=0.5, bias=tanh_bias)
                nc.vector.scalar_tensor_tensor(
                    dlt[:, sl], th[:, sl], 1.0, u[:, sl], op0=OP.add, op1=OP.mult
                )
                nc.sync.dma_start_transpose(dlt_nat[:, cs, :], dlt[:, sl])
                nc.gpsimd.tensor_add(o[:, cs, :], dlt_nat[:, cs, :], x_nat[:, cs, :])
                nc.sync.dma_start(out_r[:, cs, :], o[:, cs, :])

        # Software pipeline: emit P3(k-3), P2(k-2), P1(k-1), Load(k) per step
        # so each engine's in-order stream interleaves the samples and input
        # DMAs run a full step ahead of first use.
        ST[0] = load(0)
        for k in range(1, BPC + 3):
            if 0 <= k - 3:
                phase3(k - 3, ST[k - 3])
            if 0 <= k - 2 < BPC:
                phase2(ST[k - 2])
            if 0 <= k - 1 < BPC:
                phase1(ST[k - 1])
            if k < BPC:
                ST[k] = load(k)

    # Force Exp and Tanh to resolve to the one table set that holds both
    # (exp_and_others): contents-only lie to the set chooser, dict order
    # (= act_func_set_id) preserved; the set actually loaded at runtime does
    # contain both functions (plus Identity/Relu used by bias fallbacks).
    import concourse.bacc as bacc_mod

    real_get = bacc_mod.get_activation_tables
    target = "exp_and_others"

    def patched_get(arch):
        tabs = real_get(arch)
        strip = {AF.Exp, AF.Tanh}
        return {
            name: (set(fns) if name == target else set(fns) - strip)
            for name, fns in tabs.items()
        }

    bacc_mod.get_activation_tables = patched_get
    try:
        nc.compile()
    finally:
        bacc_mod.get_activation_tables = real_get
    return nc


def _prep_host(inputs):
    """Host-side: fold weights/biases; returns (f32 inputs, weights bf16, biases)."""
    f32 = np.float32
    g = {k: np.asarray(v, f32) for k, v in inputs.items()}

    Wo1m = g["Wo1"] - np.eye(D, dtype=f32)
    Wog2 = g["Wo"] @ g["Wg2"]                      # msg path folded into gate
    bo_msg = g["bo"] + g["bv"] @ g["Wo"]           # bv folded through Wo
    bo_uh = 0.5 * (bo_msg + g["bo1"])              # msg bias + ret bias, halved
    bo_g = bo_msg @ g["Wg2"] + g["bg1"] + g["bg2"]
    bg3h = 0.5 * g["bg3"]                          # tanh((z+bg3)/2) bias

    wmap = {
        "Wq": g["Wq"], "Wk": g["Wk"], "Wv": g["Wv"],
        "Woh": 0.5 * g["Wo"], "Wo1mh": 0.5 * Wo1m,
        "Wg1": g["Wg1"], "Wog2": Wog2, "Wg3": g["Wg3"],
    }
    bmap = {
        "bq": g["bq"], "bk": g["bk"],
        "bo_uh": bo_uh, "bo_g": bo_g, "bg3h": bg3h,
    }
    bf16 = ml_dtypes.bfloat16
    wcast = {n: np.ascontiguousarray(w.astype(bf16)) for n, w in wmap.items()}
    return g, wcast, bmap


def _prep_inputs(inputs):
    g, wcast, bmap = _prep_host(inputs)
    modes = {n: _bias_mode(v) for n, v in bmap.items()}
    base = dict(wcast)
    for n, v in bmap.items():
        if modes[n][0] == "ap":
            base[n] = np.ascontiguousarray(v.reshape(D, 1).astype(np.float32))
    x = np.ascontiguousarray(g["x"])
    xbf = np.ascontiguousarray(x.astype(ml_dtypes.bfloat16))
    in_maps = []
    for c in range(NCORES):
        m = dict(base)
        m["x"] = np.ascontiguousarray(x[c * BPC:(c + 1) * BPC])
        m["xbf"] = np.ascontiguousarray(xbf[c * BPC:(c + 1) * BPC])
        in_maps.append(m)
    return in_maps, modes


def kernel(**inputs):
    from concourse.bass_utils import run_bass_kernel_spmd

    in_maps, modes = _prep_inputs(inputs)
    key = tuple(sorted((n, k[0], k[1]) for n, k in modes.items()))
    if _CACHE.get("key") != key:
        _CACHE["nc"] = _build_nc(modes)
        _CACHE["key"] = key
    nc = _CACHE["nc"]

    res = run_bass_kernel_spmd(nc, in_maps, list(range(NCORES)))
    out = np.concatenate([r["out"] for r in res.results], axis=0)
    return out.astype(np.float32)


# revision 6
# speedup vs baseline: 1.0040x; 1.0040x over previous
"""Trainium2 Bass kernel for nn_Net_5488968204310 (gnn_message_passing).

Single-head self-attention (D=128) over N=1024 nodes + gated residual update,
batch B=32, data-parallel across 8 NeuronCores (4 samples per core).

Design notes:
  - "T layout" (features d on partitions, nodes on free dim) for every matmul;
    all eight 128x128 weight matmuls contract over d.
  - x is uploaded twice: fp32 (final residual add) and bf16 (DMA-transposed
    straight into xT; layout transposes run on the DMA xbar, not the PE).
  - QK^T: logitsT chunks [m_chunk(128) x q(1024)] = kT_chunk.T @ qT; exp() on
    the scalar engine directly from PSUM with the 1/sqrt(D) scale folded in.
  - AV keeps v as the stationary operand (few LDWEIGHTS, dense 512-col
    streams): attnT[d,q] = sum_c v_nat[c].T @ expw[c]. The softmax denominator
    is a parallel ones.T @ expw accumulation; 1/denom via the DVE
    reciprocal_approx_fast custom op (1 DVE instr, ~18 bits), freeing the
    scalar engine and making the rb write->read ordering explicit per-half.
  - gate sigmoid as 0.5*(1+tanh(z/2)): one ACT op (Tanh) instead of the
    3-op exp/ln chain. The 0.5 gate factor is folded into Wo/Wo1m/bo_u on
    the host so dlt = u_half * (tanh+1) is a single DVE scalar_tensor_tensor.
  - Every ACT function used (Exp, Tanh, Identity/Relu fallbacks) lives in the
    exp_and_others table set, enforced by a scoped patch of the table
    metadata at compile time, so there is exactly one ACT_TABLE_LOAD per run.
  - relu of the gate pre-activation runs on the (otherwise idle) Pool engine.
  - Host folds: Wo1 -> Wo1 - I (so x@(Wo1-I)+msg = ret-x directly),
    Wo@Wg2 (gate path skips msg), bv -> bo terms, bias sums, 0.5 gate factor.
"""

import math

import numpy as np
import ml_dtypes

B, N, D = 32, 1024, 128
NCORES = 8
BPC = B // NCORES  # samples per core
NT = N // 128      # node chunks per sample

_CACHE = {}


def _bias_mode(vec):
    """(kind, value) where kind in {'zero', 'uniform', 'ap'}."""
    v = np.asarray(vec, np.float32)
    if not np.any(v):
        return ("zero", 0.0)
    if np.all(v == v.flat[0]):
        return ("uniform", float(v.flat[0]))
    return ("ap", 0.0)


def _build_nc(modes):
    import concourse.bacc as bacc
    import concourse.tile as tile
    from concourse import mybir
    from contextlib import ExitStack

    f32 = mybir.dt.float32
    bf16 = mybir.dt.bfloat16
    f8 = mybir.dt.float8e4
    AF = mybir.ActivationFunctionType
    OP = mybir.AluOpType

    nc = bacc.Bacc("TRN2", target_bir_lowering=False, debug=False)

    xb_d = nc.dram_tensor("xbf", [BPC, N, D], bf16, kind="ExternalInput")
    out_d = nc.dram_tensor("out", [BPC, N, D], f32, kind="ExternalOutput")
    wnames = ["Wq", "Wk", "Wv", "Woh", "Wo1mh", "Wg1", "Wog2", "Wg3"]
    w_d = {n: nc.dram_tensor(n, [D, D], bf16, kind="ExternalInput") for n in wnames}
    b_d = {
        n: nc.dram_tensor(n, [D, 1], f32, kind="ExternalInput")
        for n in modes if modes[n][0] == "ap"
    }

    s = 1.0 / math.sqrt(D)

    with tile.TileContext(nc) as tc, ExitStack() as ctx:
        consts = ctx.enter_context(tc.tile_pool(name="consts", bufs=1))
        sb = ctx.enter_context(tc.tile_pool(name="sb", bufs=2))
        sb4 = ctx.enter_context(tc.tile_pool(name="sb4", bufs=4))
        expp = ctx.enter_context(tc.tile_pool(name="expp", bufs=2))
        pw = ctx.enter_context(tc.tile_pool(name="pw", bufs=2, space="PSUM"))
        ph = ctx.enter_context(tc.tile_pool(name="ph", bufs=2, space="PSUM"))
        pav = ctx.enter_context(tc.tile_pool(name="pav", bufs=1, space="PSUM"))
        pden = ctx.enter_context(tc.tile_pool(name="pden", bufs=1, space="PSUM"))

        W = {}
        for n in wnames:
            t = consts.tile([D, D], bf16, tag=f"w_{n}")
            nc.sync.dma_start(t, w_d[n][:, :])
            W[n] = t
        ones_dr = consts.tile([128, 2, 128], f8, tag="ones_dr")
        nc.vector.memset(ones_dr, 1.0)
        expbias = consts.tile([128, 1], f32, tag="expbias")
        nc.vector.memset(expbias, -2.0)
        BV = {}
        for n in b_d:
            t = consts.tile([D, 1], f32, tag=f"b_{n}")
            nc.sync.dma_start(t, b_d[n][:, :])
            BV[n] = t
        for n, (kind, val) in modes.items():
            if kind == "uniform":
                t = consts.tile([D, 1], f32, tag=f"b_{n}")
                nc.vector.memset(t, val)
                BV[n] = t

        def copyback(dst, src, bname, engine_copy):
            """psum->sbuf copy honoring the bias mode for `bname`."""
            kind, val = modes[bname]
            if kind == "zero":
                engine_copy(dst, src)
            else:
                nc.scalar.activation(dst, src, AF.Identity, bias=BV[bname])

        ST = {}

        def load(b):
            """input DMAs for sample b (issued one pipeline step early)."""
            st = {}
            x_nat = sb4.tile([128, NT, D], bf16, tag="x_nat")
            nc.sync.dma_start(x_nat, xb_d[b].rearrange("(c p) d -> p c d", p=128))
            xT = sb4.tile([128, N], bf16, tag="xT")  # [d, n]
            nc.sync.dma_start_transpose(xT, xb_d[b])
            st["x_nat"], st["xT"] = x_nat, xT
            return st

        def phase1(st):
            """q/k/v projections, QK^T + exp."""
            xT = st["xT"]

            p_q = pw.tile([128, N], f32, tag="pw")
            nc.tensor.matmul(p_q[:, 0:512], W["Wq"], xT[:, 0:512], start=True, stop=True)
            nc.tensor.matmul(p_q[:, 512:1024], W["Wq"], xT[:, 512:1024], start=True, stop=True)
            qT = sb.tile([128, N], bf16, tag="qT")
            copyback(qT, p_q, "bq", nc.vector.tensor_copy)

            p_k = pw.tile([128, N], f32, tag="pw")
            nc.tensor.matmul(p_k[:, 0:512], W["Wk"], xT[:, 0:512], start=True, stop=True)
            nc.tensor.matmul(p_k[:, 512:1024], W["Wk"], xT[:, 512:1024], start=True, stop=True)
            kT = sb.tile([128, N], bf16, tag="kT")
            copyback(kT, p_k, "bk", nc.vector.tensor_copy)

            p_v = pw.tile([128, N], f32, tag="pw")
            for c in range(NT):
                nc.tensor.matmul(p_v[:, c * 128:(c + 1) * 128], xT[:, c * 128:(c + 1) * 128], W["Wv"], start=True, stop=True)
            v_nat = sb.tile([128, NT, 128], f8, tag="v_nat")
            nc.vector.tensor_copy(v_nat, p_v.rearrange("p (c n) -> p c n", c=NT))
            st["v_nat"] = v_nat

            # exp output in fp8e4m3: bias -2 rescales exp into fp8 range; the
            # uniform factor e^-2 cancels between numerator and denominator.
            expw = expp.tile([128, NT, N], f8, tag="expw")  # [m', c_m, q]
            for c in range(NT):
                p_l = pw.tile([128, N], f32, tag="pw")
                kTc = kT[:, c * 128:(c + 1) * 128]
                nc.tensor.matmul(p_l[:, 0:512], kTc, qT[:, 0:512], start=True, stop=True)
                nc.tensor.matmul(p_l[:, 512:1024], kTc, qT[:, 512:1024], start=True, stop=True)
                nc.scalar.activation(expw[:, c, :], p_l, AF.Exp, scale=s, bias=expbias)
            st["expw"] = expw

        def phase23(b, st):
            """softmax normalize + gated update tail; store.

            u = 0.5*(ret - x) comes straight out of the halved-weight matmul;
            gate = 0.5*(1 + tanh((z+bg3)/2)); out = x + u*(1+tanh)."""
            expw, v_nat = st["expw"], st["v_nat"]
            x_nat, xT = st["x_nat"], st["xT"]
            rb = sb.tile([128, N], f32, tag="rb")
            attnT = sb.tile([128, N], bf16, tag="attnT")
            u = sb.tile([128, N], f32, tag="u")
            gp = sb.tile([128, N], bf16, tag="gp")
            th = sb.tile([128, N], bf16, tag="th")
            dlt = sb.tile([128, N], bf16, tag="dlt")
            dlt_nat = sb.tile([128, NT, 128], bf16, tag="dlt_nat")
            o = sb.tile([128, NT, D], f32, tag="o")
            out_r = out_d[b].rearrange("(c p) d -> p c d", p=128)
            H = NT // 2
            tanh_bias = BV["bg3h"] if "bg3h" in BV else 0.0
            for h in range(2):
                sl = slice(h * 512, (h + 1) * 512)
                p_dn = pden.tile([128, 512], f32, tag="pden")
                for c in range(NT // 2):
                    nc.tensor.matmul(
                        p_dn, ones_dr, expw[:, 2 * c:2 * c + 2, sl],
                        start=(c == 0), stop=(c == NT // 2 - 1),
                        perf_mode=mybir.MatmulPerfMode.DoubleRow,
                    )
                nc.vector.reciprocal_approx_fast(rb[:, sl], p_dn)
                p_av = pav.tile([128, 512], f32, tag="pav")
                for c in range(NT // 2):
                    nc.tensor.matmul(
                        p_av, v_nat[:, 2 * c:2 * c + 2, :], expw[:, 2 * c:2 * c + 2, sl],
                        start=(c == 0), stop=(c == NT // 2 - 1),
                        perf_mode=mybir.MatmulPerfMode.DoubleRow,
                    )
                nc.vector.tensor_mul(attnT[:, sl], p_av, rb[:, sl])
            for h in range(2):
                sl = slice(h * 512, (h + 1) * 512)
                cs = slice(h * H, (h + 1) * H)

                p_m = ph.tile([128, 512], f32, tag="pwh")
                nc.tensor.matmul(p_m, W["Woh"], attnT[:, sl], start=True, stop=False)
                nc.tensor.matmul(p_m, W["Wo1mh"], xT[:, sl], start=False, stop=True)
                copyback(u[:, sl], p_m, "bo_uh", nc.vector.tensor_copy)

                p_g = ph.tile([128, 512], f32, tag="pwh")
                nc.tensor.matmul(p_g, W["Wg1"], xT[:, sl], start=True, stop=False)
                nc.tensor.matmul(p_g, W["Wog2"], attnT[:, sl], start=False, stop=True)
                if modes["bo_g"][0] == "zero":
                    nc.vector.tensor_scalar(gp[:, sl], p_g, 0.0, None, op0=OP.max)
                else:
                    nc.scalar.activation(gp[:, sl], p_g, AF.Relu, bias=BV["bo_g"])

                p_g3 = ph.tile([128, 512], f32, tag="pwh")
                nc.tensor.matmul(p_g3, W["Wg3"], gp[:, sl], start=True, stop=True)
                nc.scalar.activation(th[:, sl], p_g3, AF.Tanh, scale=0.5, bias=tanh_bias)
                nc.vector.scalar_tensor_tensor(
                    dlt[:, sl], th[:, sl], 1.0, u[:, sl], op0=OP.add, op1=OP.mult
                )
                nc.sync.dma_start_transpose(dlt_nat[:, cs, :], dlt[:, sl])
                nc.gpsimd.tensor_add(o[:, cs, :], dlt_nat[:, cs, :], x_nat[:, cs, :])
                nc.sync.dma_start(out_r[:, cs, :], o[:, cs, :])

        # Software pipeline: emit P23(k-2), P1(k-1), Load(k) per step so each
        # engine\'s in-order stream interleaves two samples and input DMAs run
        # a full step ahead of first use.
        ST[0] = load(0)
        for k in range(1, BPC + 2):
            if 0 <= k - 2:
                phase23(k - 2, ST[k - 2])
            if 0 <= k - 1 < BPC:
                phase1(ST[k - 1])
            if k < BPC:
                ST[k] = load(k)

    # Force Exp and Tanh to resolve to the one table set that holds both
    # (exp_and_others): contents-only lie to the set chooser, dict order
    # (= act_func_set_id) preserved; the set actually loaded at runtime does
    # contain both functions (plus Identity/Relu used by bias fallbacks).
    import concourse.bacc as bacc_mod

    real_get = bacc_mod.get_activation_tables
    target = "exp_and_others"

    def patched_get(arch):
        tabs = real_get(arch)
        strip = {AF.Exp, AF.Tanh}
        return {
            name: (set(fns) if name == target else set(fns) - strip)
            for name, fns in tabs.items()
        }

    bacc_mod.get_activation_tables = patched_get
    try:
        nc.compile()
    finally:
        bacc_mod.get_activation_tables = real_get
    return nc


def _prep_host(inputs):
    """Host-side: fold weights/biases; returns (f32 inputs, weights bf16, biases)."""
    f32 = np.float32
    g = {k: np.asarray(v, f32) for k, v in inputs.items()}

    Wo1m = g["Wo1"] - np.eye(D, dtype=f32)
    Wog2 = g["Wo"] @ g["Wg2"]                      # msg path folded into gate
    bo_msg = g["bo"] + g["bv"] @ g["Wo"]           # bv folded through Wo
    bo_uh = 0.5 * (bo_msg + g["bo1"])              # msg bias + ret bias, halved
    bo_g = bo_msg @ g["Wg2"] + g["bg1"] + g["bg2"]
    bg3h = 0.5 * g["bg3"]                          # tanh((z+bg3)/2) bias

    wmap = {
        "Wq": g["Wq"], "Wk": g["Wk"], "Wv": g["Wv"],
        "Woh": 0.5 * g["Wo"], "Wo1mh": 0.5 * Wo1m,
        "Wg1": g["Wg1"], "Wog2": Wog2, "Wg3": g["Wg3"],
    }
    bmap = {
        "bq": g["bq"], "bk": g["bk"],
        "bo_uh": bo_uh, "bo_g": bo_g, "bg3h": bg3h,
    }
    bf16 = ml_dtypes.bfloat16
    wcast = {n: np.ascontiguousarray(w.astype(bf16)) for n, w in wmap.items()}
    return g, wcast, bmap


def _prep_inputs(inputs):
    g, wcast, bmap = _prep_host(inputs)
    modes = {n: _bias_mode(v) for n, v in bmap.items()}
    base = dict(wcast)
    for n, v in bmap.items():
        if modes[n][0] == "ap":
            base[n] = np.ascontiguousarray(v.reshape(D, 1).astype(np.float32))
    xbf = np.ascontiguousarray(g["x"].astype(ml_dtypes.bfloat16))
    in_maps = []
    for c in range(NCORES):
        m = dict(base)
        m["xbf"] = np.ascontiguousarray(xbf[c * BPC:(c + 1) * BPC])
        in_maps.append(m)
    return in_maps, modes


def kernel(**inputs):
    from concourse.bass_utils import run_bass_kernel_spmd

    in_maps, modes = _prep_inputs(inputs)
    key = tuple(sorted((n, k[0], k[1]) for n, k in modes.items()))
    if _CACHE.get("key") != key:
        _CACHE["nc"] = _build_nc(modes)
        _CACHE["key"] = key
    nc = _CACHE["nc"]

    res = run_bass_kernel_spmd(nc, in_maps, list(range(NCORES)))
    out = np.concatenate([r["out"] for r in res.results], axis=0)
    return out.astype(np.float32)


# revision 8
# speedup vs baseline: 1.1186x; 1.1141x over previous
"""Trainium2 Bass kernel for nn_Net_5488968204310 (gnn_message_passing).

Single-head self-attention (D=128) over N=1024 nodes + gated residual update,
batch B=32, data-parallel across 8 NeuronCores (4 samples per core).

Design notes:
  - "T layout" (features d on partitions, nodes on free dim) for every matmul;
    contraction is always over d.
  - QK fold: logit_ij = x_i.(M^T x_j) with M = Wq@Wk^T folded on the host, so
    only ONE projection matmul (yT = (Wk Wq^T)^T... stationary S=Wk@Wq^T) is
    needed and the logits' moving operand is xT itself. The per-query bias
    term is softmax-invariant and dropped; the per-key term folds into y's
    bias (Wk@bq).
  - V fold: attn@Wo = (A@X)@(Wv@Wo). Attention runs over raw X (host-cast
    fp8 upload, natural layout) and Wv folds into the tail weights, removing
    the v projection and its PSUM->SBUF cast.
  - QK^T: logitsT chunks [m_chunk(128) x q(1024)] = yT_chunk.T @ xT; exp() on
    the scalar engine straight from PSUM with the 1/sqrt(D) scale and a -2
    bias folded in (rescales exp into fp8 range; the uniform e^-2 factor
    cancels between numerator and denominator).
  - A@X keeps xf8 as the stationary operand (DoubleRow fp8, dense 512-col
    streams). The softmax denominator is a parallel ones.T @ expw DoubleRow
    accumulation; 1/denom via the DVE reciprocal_approx_fast custom op.
  - gate sigmoid as 0.5*(1+tanh(z/2)): one ACT op (Tanh) instead of a 3-op
    exp/ln chain. The 0.5 gate factor is folded into the u-path weights on
    the host so dlt = u_half * (tanh+1) is a single DVE scalar_tensor_tensor.
  - ACT functions used (Exp, Tanh, Identity/Relu fallbacks) all live in the
    exp_and_others table set, enforced by a scoped patch of the table
    metadata at compile time: exactly one ACT_TABLE_LOAD per run.
  - The residual add runs in bf16 on the Pool engine (x loaded bf16); all
    six folded 128x128 weights ship as ONE packed DMA to cut issue latency.
"""

import math

import numpy as np
import ml_dtypes

B, N, D = 32, 1024, 128
NCORES = 8
BPC = B // NCORES  # samples per core
NT = N // 128      # node chunks per sample

WNAMES = ["Wm", "Woh", "Wo1mh", "Wg1", "Wog2", "Wg3"]

_CACHE = {}


def _bias_mode(vec):
    """(kind, value) where kind in {'zero', 'uniform', 'ap'}."""
    v = np.asarray(vec, np.float32)
    if not np.any(v):
        return ("zero", 0.0)
    if np.all(v == v.flat[0]):
        return ("uniform", float(v.flat[0]))
    return ("ap", 0.0)


def _build_nc(modes):
    import concourse.bacc as bacc
    import concourse.tile as tile
    from concourse import mybir
    from contextlib import ExitStack

    f32 = mybir.dt.float32
    bf16 = mybir.dt.bfloat16
    f8 = mybir.dt.float8e4
    AF = mybir.ActivationFunctionType
    OP = mybir.AluOpType

    nc = bacc.Bacc("TRN2", target_bir_lowering=False, debug=False)

    xb_d = nc.dram_tensor("xbf", [BPC, N, D], bf16, kind="ExternalInput")
    xf8_d = nc.dram_tensor("xf8", [BPC, N, D], f8, kind="ExternalInput")
    out_d = nc.dram_tensor("out", [BPC, N, D], f32, kind="ExternalOutput")
    wp_d = nc.dram_tensor("wpack", [D, len(WNAMES), D], bf16, kind="ExternalInput")
    b_d = {
        n: nc.dram_tensor(n, [D, 1], f32, kind="ExternalInput")
        for n in modes if modes[n][0] == "ap"
    }

    s = 1.0 / math.sqrt(D)

    with tile.TileContext(nc) as tc, ExitStack() as ctx:
        consts = ctx.enter_context(tc.tile_pool(name="consts", bufs=1))
        sb = ctx.enter_context(tc.tile_pool(name="sb", bufs=2))
        sb3 = ctx.enter_context(tc.tile_pool(name="sb3", bufs=3))
        expp = ctx.enter_context(tc.tile_pool(name="expp", bufs=2))
        pw = ctx.enter_context(tc.tile_pool(name="pw", bufs=2, space="PSUM"))
        ph = ctx.enter_context(tc.tile_pool(name="ph", bufs=2, space="PSUM"))
        pav = ctx.enter_context(tc.tile_pool(name="pav", bufs=1, space="PSUM"))
        pden = ctx.enter_context(tc.tile_pool(name="pden", bufs=1, space="PSUM"))

        ST = {}

        def load(b):
            """input DMAs for sample b (issued one pipeline step early)."""
            st = {}
            x_nat = sb3.tile([128, NT, D], bf16, tag="x_nat")
            nc.sync.dma_start(x_nat, xb_d[b].rearrange("(c p) d -> p c d", p=128))
            xf8_nat = sb3.tile([128, NT, D], f8, tag="xf8_nat")
            nc.sync.dma_start(xf8_nat, xf8_d[b].rearrange("(c p) d -> p c d", p=128))
            xT = sb3.tile([128, N], bf16, tag="xT")  # [d, n]
            nc.sync.dma_start_transpose(xT, xb_d[b])
            st["x_nat"], st["xf8_nat"], st["xT"] = x_nat, xf8_nat, xT
            return st

        # input DMAs for sample 0 go out before anything else; the packed
        # weight DMA + consts follow on other engines so nothing serializes
        # behind the sequencer's per-DMA issue cost.
        ST[0] = load(0)

        wpack = consts.tile([D, len(WNAMES), D], bf16, tag="wpack")
        nc.scalar.dma_start(wpack, wp_d[:, :, :])
        W = {n: wpack[:, i, :] for i, n in enumerate(WNAMES)}
        ones_dr = consts.tile([128, 2, 128], f8, tag="ones_dr")
        nc.gpsimd.memset(ones_dr, 1.0)
        expbias = consts.tile([128, 1], f32, tag="expbias")
        nc.gpsimd.memset(expbias, -2.0)
        BV = {}
        for n in b_d:
            t = consts.tile([D, 1], f32, tag=f"b_{n}")
            nc.scalar.dma_start(t, b_d[n][:, :])
            BV[n] = t
        for n, (kind, val) in modes.items():
            if kind == "uniform":
                t = consts.tile([D, 1], f32, tag=f"b_{n}")
                nc.gpsimd.memset(t, val)
                BV[n] = t

        def copyback(dst, src, bname, engine_copy):
            """psum->sbuf copy honoring the bias mode for `bname`."""
            kind, val = modes[bname]
            if kind == "zero":
                engine_copy(dst, src)
            else:
                nc.scalar.activation(dst, src, AF.Identity, bias=BV[bname])

        def phase1(st):
            """y projection, QK^T + exp."""
            xT = st["xT"]

            p_y = pw.tile([128, N], f32, tag="pw")
            nc.tensor.matmul(p_y[:, 0:512], W["Wm"], xT[:, 0:512], start=True, stop=True)
            nc.tensor.matmul(p_y[:, 512:1024], W["Wm"], xT[:, 512:1024], start=True, stop=True)
            yT = sb.tile([128, N], bf16, tag="yT")
            copyback(yT, p_y, "by", nc.vector.tensor_copy)

            expw = expp.tile([128, NT, N], f8, tag="expw")  # [m', c_m, q]
            for c in range(NT):
                p_l = pw.tile([128, N], f32, tag="pw")
                yTc = yT[:, c * 128:(c + 1) * 128]
                nc.tensor.matmul(p_l[:, 0:512], yTc, xT[:, 0:512], start=True, stop=True)
                nc.tensor.matmul(p_l[:, 512:1024], yTc, xT[:, 512:1024], start=True, stop=True)
                nc.scalar.activation(expw[:, c, :], p_l, AF.Exp, scale=s, bias=expbias)
            st["expw"] = expw

        def phase23(b, st):
            """softmax normalize + gated update tail; store.

            u = 0.5*(ret - x) comes straight out of the folded-weight matmul;
            gate = 0.5*(1 + tanh((z+bg3)/2)); out = x + u*(1+tanh)."""
            expw, xf8_nat = st["expw"], st["xf8_nat"]
            x_nat, xT = st["x_nat"], st["xT"]
            rb = sb.tile([128, N], f32, tag="rb")
            attnT = sb.tile([128, N], bf16, tag="attnT")
            u = sb.tile([128, N], f32, tag="u")
            gp = sb.tile([128, N], bf16, tag="gp")
            th = sb.tile([128, N], bf16, tag="th")
            dlt = sb.tile([128, N], bf16, tag="dlt")
            dlt_nat = sb.tile([128, NT, 128], bf16, tag="dlt_nat")
            o = sb.tile([128, NT, D], f32, tag="o")
            out_r = out_d[b].rearrange("(c p) d -> p c d", p=128)
            H = NT // 2
            tanh_bias = BV["bg3h"] if "bg3h" in BV else 0.0
            for h in range(2):
                sl = slice(h * 512, (h + 1) * 512)
                p_dn = pden.tile([128, 512], f32, tag="pden")
                for c in range(NT // 2):
                    nc.tensor.matmul(
                        p_dn, ones_dr, expw[:, 2 * c:2 * c + 2, sl],
                        start=(c == 0), stop=(c == NT // 2 - 1),
                        perf_mode=mybir.MatmulPerfMode.DoubleRow,
                    )
                nc.vector.reciprocal_approx_fast(rb[:, sl], p_dn)
                p_av = pav.tile([128, 512], f32, tag="pav")
                for c in range(NT // 2):
                    nc.tensor.matmul(
                        p_av, xf8_nat[:, 2 * c:2 * c + 2, :], expw[:, 2 * c:2 * c + 2, sl],
                        start=(c == 0), stop=(c == NT // 2 - 1),
                        perf_mode=mybir.MatmulPerfMode.DoubleRow,
                    )
                nc.vector.tensor_mul(attnT[:, sl], p_av, rb[:, sl])
            for h in range(2):
                sl = slice(h * 512, (h + 1) * 512)
                cs = slice(h * H, (h + 1) * H)

                p_m = ph.tile([128, 512], f32, tag="pwh")
                nc.tensor.matmul(p_m, W["Woh"], attnT[:, sl], start=True, stop=False)
                nc.tensor.matmul(p_m, W["Wo1mh"], xT[:, sl], start=False, stop=True)
                copyback(u[:, sl], p_m, "bo_uh", nc.vector.tensor_copy)

                p_g = ph.tile([128, 512], f32, tag="pwh")
                nc.tensor.matmul(p_g, W["Wg1"], xT[:, sl], start=True, stop=False)
                nc.tensor.matmul(p_g, W["Wog2"], attnT[:, sl], start=False, stop=True)
                if modes["bo_g"][0] == "zero":
                    nc.vector.tensor_scalar(gp[:, sl], p_g, 0.0, None, op0=OP.max)
                else:
                    nc.scalar.activation(gp[:, sl], p_g, AF.Relu, bias=BV["bo_g"])

                p_g3 = ph.tile([128, 512], f32, tag="pwh")
                nc.tensor.matmul(p_g3, W["Wg3"], gp[:, sl], start=True, stop=True)
                nc.scalar.activation(th[:, sl], p_g3, AF.Tanh, scale=0.5, bias=tanh_bias)
                nc.vector.scalar_tensor_tensor(
                    dlt[:, sl], th[:, sl], 1.0, u[:, sl], op0=OP.add, op1=OP.mult
                )
                nc.sync.dma_start_transpose(dlt_nat[:, cs, :], dlt[:, sl])
                nc.gpsimd.tensor_add(o[:, cs, :], dlt_nat[:, cs, :], x_nat[:, cs, :])
                nc.sync.dma_start(out_r[:, cs, :], o[:, cs, :])

        # Software pipeline: emit P23(k-2), P1(k-1), Load(k) per step so each
        # engine's in-order stream interleaves two samples and input DMAs run
        # a full step ahead of first use.
        for k in range(1, BPC + 2):
            if 0 <= k - 2:
                phase23(k - 2, ST[k - 2])
            if 0 <= k - 1 < BPC:
                phase1(ST[k - 1])
            if k < BPC:
                ST[k] = load(k)

    # Force Exp and Tanh to resolve to the one table set that holds both
    # (exp_and_others): contents-only lie to the set chooser, dict order
    # (= act_func_set_id) preserved; the set actually loaded at runtime does
    # contain both functions (plus Identity/Relu used by bias fallbacks).
    import concourse.bacc as bacc_mod

    real_get = bacc_mod.get_activation_tables
    target = "exp_and_others"

    def patched_get(arch):
        tabs = real_get(arch)
        strip = {AF.Exp, AF.Tanh}
        return {
            name: (set(fns) if name == target else set(fns) - strip)
            for name, fns in tabs.items()
        }

    bacc_mod.get_activation_tables = patched_get
    try:
        nc.compile()
    finally:
        bacc_mod.get_activation_tables = real_get
    return nc


def _prep_host(inputs):
    """Host-side: fold weights/biases; returns (f32 inputs, wpack bf16, biases)."""
    f32 = np.float32
    g = {k: np.asarray(v, f32) for k, v in inputs.items()}

    Wm = g["Wk"] @ g["Wq"].T                       # y = x@Wk@Wq^T; logit=x_i.y_j
    Wvo = g["Wv"] @ g["Wo"]                        # v path folded into tail
    Wo1m = g["Wo1"] - np.eye(D, dtype=f32)
    Wog2 = Wvo @ g["Wg2"]                          # msg path folded into gate
    bo_msg = g["bo"] + g["bv"] @ g["Wo"]           # bv folded through Wo
    bo_uh = 0.5 * (bo_msg + g["bo1"])              # msg bias + ret bias, halved
    bo_g = bo_msg @ g["Wg2"] + g["bg1"] + g["bg2"]
    bg3h = 0.5 * g["bg3"]                          # tanh((z+bg3)/2) bias
    by = g["Wk"] @ g["bq"]                         # per-key logit bias

    wmap = {
        "Wm": Wm, "Woh": 0.5 * Wvo, "Wo1mh": 0.5 * Wo1m,
        "Wg1": g["Wg1"], "Wog2": Wog2, "Wg3": g["Wg3"],
    }
    bmap = {
        "by": by,
        "bo_uh": bo_uh, "bo_g": bo_g, "bg3h": bg3h,
    }
    bf16 = ml_dtypes.bfloat16
    wpack = np.stack([wmap[n] for n in WNAMES], axis=1).astype(bf16)
    return g, np.ascontiguousarray(wpack), bmap


def _prep_inputs(inputs):
    g, wpack, bmap = _prep_host(inputs)
    modes = {n: _bias_mode(v) for n, v in bmap.items()}
    base = {"wpack": wpack}
    for n, v in bmap.items():
        if modes[n][0] == "ap":
            base[n] = np.ascontiguousarray(v.reshape(D, 1).astype(np.float32))
    xbf = np.ascontiguousarray(g["x"].astype(ml_dtypes.bfloat16))
    xf8 = np.ascontiguousarray(g["x"].astype(ml_dtypes.float8_e4m3fn))
    in_maps = []
    for c in range(NCORES):
        m = dict(base)
        m["xbf"] = np.ascontiguousarray(xbf[c * BPC:(c + 1) * BPC])
        m["xf8"] = np.ascontiguousarray(xf8[c * BPC:(c + 1) * BPC])
        in_maps.append(m)
    return in_maps, modes


def kernel(**inputs):
    from concourse.bass_utils import run_bass_kernel_spmd

    in_maps, modes = _prep_inputs(inputs)
    key = tuple(sorted((n, k[0], k[1]) for n, k in modes.items()))
    if _CACHE.get("key") != key:
        _CACHE["nc"] = _build_nc(modes)
        _CACHE["key"] = key
    nc = _CACHE["nc"]

    res = run_bass_kernel_spmd(nc, in_maps, list(range(NCORES)))
    out = np.concatenate([r["out"] for r in res.results], axis=0)
    return out.astype(np.float32)


# revision 9
# speedup vs baseline: 1.1690x; 1.0451x over previous
"""Trainium2 Bass kernel for nn_Net_5488968204310 (gnn_message_passing).

Single-head self-attention (D=128) over N=1024 nodes + gated residual update,
batch B=32, data-parallel across 8 NeuronCores (4 samples per core).

Design notes:
  - "T layout" (features d on partitions, nodes on free dim) for every matmul;
    contraction is always over d.
  - QK fold: logit_ij = x_i.(M^T x_j) with M = Wq@Wk^T folded on the host, so
    only ONE projection matmul (yT = (Wk Wq^T)^T... stationary S=Wk@Wq^T) is
    needed and the logits' moving operand is xT itself. The per-query bias
    term is softmax-invariant and dropped; the per-key term folds into y's
    bias (Wk@bq).
  - V fold: attn@Wo = (A@X)@(Wv@Wo). Attention runs over raw X (host-cast
    fp8 upload, natural layout) and Wv folds into the tail weights, removing
    the v projection and its PSUM->SBUF cast.
  - QK^T: logitsT chunks [m_chunk(128) x q(1024)] = yT_chunk.T @ xT; exp() on
    the scalar engine straight from PSUM with the 1/sqrt(D) scale and a -2
    bias folded in (rescales exp into fp8 range; the uniform e^-2 factor
    cancels between numerator and denominator).
  - A@X keeps xf8 as the stationary operand (DoubleRow fp8, dense 512-col
    streams). The softmax denominator is a parallel ones.T @ expw DoubleRow
    accumulation; 1/denom via the DVE reciprocal_approx_fast custom op.
  - gate sigmoid as 0.5*(1+tanh(z/2)): one ACT op (Tanh) instead of a 3-op
    exp/ln chain. The 0.5 gate factor is folded into the u-path weights on
    the host so dlt = u_half * (tanh+1) is a single DVE scalar_tensor_tensor.
  - ACT functions used (Exp, Tanh, Identity/Relu fallbacks) all live in the
    exp_and_others table set, enforced by a scoped patch of the table
    metadata at compile time: exactly one ACT_TABLE_LOAD per run.
  - The residual add runs in bf16 on the Pool engine (x loaded bf16); all
    six folded 128x128 weights ship as ONE packed DMA to cut issue latency.
"""

import math

import numpy as np
import ml_dtypes

B, N, D = 32, 1024, 128
NCORES = 8
BPC = B // NCORES  # samples per core
NT = N // 128      # node chunks per sample

WNAMES = ["Wm", "Woh", "Wo1mh", "Wg1", "Wog2", "Wg3"]

_CACHE = {}


def _bias_mode(vec):
    """(kind, value) where kind in {'zero', 'uniform', 'ap'}."""
    v = np.asarray(vec, np.float32)
    if not np.any(v):
        return ("zero", 0.0)
    if np.all(v == v.flat[0]):
        return ("uniform", float(v.flat[0]))
    return ("ap", 0.0)


def _build_nc(modes):
    import concourse.bacc as bacc
    import concourse.tile as tile
    from concourse import mybir
    from contextlib import ExitStack

    f32 = mybir.dt.float32
    bf16 = mybir.dt.bfloat16
    f8 = mybir.dt.float8e4
    AF = mybir.ActivationFunctionType
    OP = mybir.AluOpType

    nc = bacc.Bacc("TRN2", target_bir_lowering=False, debug=False)

    xb_d = nc.dram_tensor("xbf", [BPC, N, D], bf16, kind="ExternalInput")
    xf8_d = nc.dram_tensor("xf8", [BPC, N, D], f8, kind="ExternalInput")
    out_d = nc.dram_tensor("out", [BPC, N, D], f32, kind="ExternalOutput")
    wp_d = nc.dram_tensor("wpack", [D, len(WNAMES), D], bf16, kind="ExternalInput")
    b_d = {
        n: nc.dram_tensor(n, [D, 1], f32, kind="ExternalInput")
        for n in modes if modes[n][0] == "ap"
    }

    s = 1.0 / math.sqrt(D)

    with tile.TileContext(nc) as tc, ExitStack() as ctx:
        consts = ctx.enter_context(tc.tile_pool(name="consts", bufs=1))
        sb = ctx.enter_context(tc.tile_pool(name="sb", bufs=2))
        sb3 = ctx.enter_context(tc.tile_pool(name="sb3", bufs=3))
        expp = ctx.enter_context(tc.tile_pool(name="expp", bufs=2))
        pw = ctx.enter_context(tc.tile_pool(name="pw", bufs=2, space="PSUM"))
        ph = ctx.enter_context(tc.tile_pool(name="ph", bufs=2, space="PSUM"))
        pav = ctx.enter_context(tc.tile_pool(name="pav", bufs=1, space="PSUM"))
        pden = ctx.enter_context(tc.tile_pool(name="pden", bufs=1, space="PSUM"))

        ST = {}

        def load(b):
            """input DMAs for sample b (issued one pipeline step early)."""
            st = {}
            xT = sb3.tile([128, N], bf16, tag="xT")  # [d, n]
            nc.sync.dma_start_transpose(xT, xb_d[b])
            x_nat = sb3.tile([128, NT, D], bf16, tag="x_nat")
            nc.sync.dma_start(x_nat, xb_d[b].rearrange("(c p) d -> p c d", p=128))
            xf8_nat = sb3.tile([128, NT, D], f8, tag="xf8_nat")
            nc.sync.dma_start(xf8_nat, xf8_d[b].rearrange("(c p) d -> p c d", p=128))
            st["x_nat"], st["xf8_nat"], st["xT"] = x_nat, xf8_nat, xT
            return st

        # input DMAs for sample 0 go out before anything else; the packed
        # weight DMA + consts follow on other engines so nothing serializes
        # behind the sequencer's per-DMA issue cost.
        ST[0] = load(0)

        wpack = consts.tile([D, len(WNAMES), D], bf16, tag="wpack")
        nc.scalar.dma_start(wpack, wp_d[:, :, :])
        W = {n: wpack[:, i, :] for i, n in enumerate(WNAMES)}
        ones_dr = consts.tile([128, 2, 128], f8, tag="ones_dr")
        nc.gpsimd.memset(ones_dr, 1.0)
        expbias = consts.tile([128, 1], f32, tag="expbias")
        nc.gpsimd.memset(expbias, -2.0)
        BV = {}
        for n in b_d:
            t = consts.tile([D, 1], f32, tag=f"b_{n}")
            nc.scalar.dma_start(t, b_d[n][:, :])
            BV[n] = t
        for n, (kind, val) in modes.items():
            if kind == "uniform":
                t = consts.tile([D, 1], f32, tag=f"b_{n}")
                nc.gpsimd.memset(t, val)
                BV[n] = t

        def copyback(dst, src, bname, engine_copy):
            """psum->sbuf copy honoring the bias mode for `bname`."""
            kind, val = modes[bname]
            if kind == "zero":
                engine_copy(dst, src)
            else:
                nc.scalar.activation(dst, src, AF.Identity, bias=BV[bname])

        def phase1(st):
            """y projection, QK^T + exp."""
            xT = st["xT"]

            p_y = pw.tile([128, N], f32, tag="pw")
            nc.tensor.matmul(p_y[:, 0:512], W["Wm"], xT[:, 0:512], start=True, stop=True)
            nc.tensor.matmul(p_y[:, 512:1024], W["Wm"], xT[:, 512:1024], start=True, stop=True)
            yT = sb.tile([128, N], bf16, tag="yT")
            copyback(yT, p_y, "by", nc.vector.tensor_copy)

            expw = expp.tile([128, NT, N], f8, tag="expw")  # [m', c_m, q]
            for c in range(NT):
                p_l = pw.tile([128, N], f32, tag="pw")
                yTc = yT[:, c * 128:(c + 1) * 128]
                nc.tensor.matmul(p_l[:, 0:512], yTc, xT[:, 0:512], start=True, stop=True)
                nc.tensor.matmul(p_l[:, 512:1024], yTc, xT[:, 512:1024], start=True, stop=True)
                nc.scalar.activation(expw[:, c, :], p_l, AF.Exp, scale=s, bias=expbias)
            st["expw"] = expw

        def phase23(b, st):
            """softmax normalize + gated update tail; store.

            u = 0.5*(ret - x) comes straight out of the folded-weight matmul;
            gate = 0.5*(1 + tanh((z+bg3)/2)); out = x + u*(1+tanh)."""
            expw, xf8_nat = st["expw"], st["xf8_nat"]
            x_nat, xT = st["x_nat"], st["xT"]
            rb = sb.tile([128, N], f32, tag="rb")
            attnT = sb.tile([128, N], bf16, tag="attnT")
            u = sb.tile([128, N], f32, tag="u")
            gp = sb.tile([128, N], bf16, tag="gp")
            th = sb.tile([128, N], bf16, tag="th")
            dlt = sb.tile([128, N], bf16, tag="dlt")
            dlt_nat = sb.tile([128, NT, 128], bf16, tag="dlt_nat")
            o = sb.tile([128, NT, D], f32, tag="o")
            out_r = out_d[b].rearrange("(c p) d -> p c d", p=128)
            H = NT // 2
            tanh_bias = BV["bg3h"] if "bg3h" in BV else 0.0
            for h in range(2):
                sl = slice(h * 512, (h + 1) * 512)
                p_dn = pden.tile([128, 512], f32, tag="pden")
                for c in range(NT // 2):
                    nc.tensor.matmul(
                        p_dn, ones_dr, expw[:, 2 * c:2 * c + 2, sl],
                        start=(c == 0), stop=(c == NT // 2 - 1),
                        perf_mode=mybir.MatmulPerfMode.DoubleRow,
                    )
                nc.vector.reciprocal_approx_fast(rb[:, sl], p_dn)
                p_av = pav.tile([128, 512], f32, tag="pav")
                for c in range(NT // 2):
                    nc.tensor.matmul(
                        p_av, xf8_nat[:, 2 * c:2 * c + 2, :], expw[:, 2 * c:2 * c + 2, sl],
                        start=(c == 0), stop=(c == NT // 2 - 1),
                        perf_mode=mybir.MatmulPerfMode.DoubleRow,
                    )
                nc.vector.tensor_mul(attnT[:, sl], p_av, rb[:, sl])
            for h in range(2):
                sl = slice(h * 512, (h + 1) * 512)
                cs = slice(h * H, (h + 1) * H)

                p_m = ph.tile([128, 512], f32, tag="pwh")
                nc.tensor.matmul(p_m, W["Woh"], attnT[:, sl], start=True, stop=False)
                nc.tensor.matmul(p_m, W["Wo1mh"], xT[:, sl], start=False, stop=True)
                copyback(u[:, sl], p_m, "bo_uh", nc.vector.tensor_copy)

                p_g = ph.tile([128, 512], f32, tag="pwh")
                nc.tensor.matmul(p_g, W["Wg1"], xT[:, sl], start=True, stop=False)
                nc.tensor.matmul(p_g, W["Wog2"], attnT[:, sl], start=False, stop=True)
                if modes["bo_g"][0] == "zero":
                    nc.vector.tensor_scalar(gp[:, sl], p_g, 0.0, None, op0=OP.max)
                else:
                    nc.scalar.activation(gp[:, sl], p_g, AF.Relu, bias=BV["bo_g"])

                p_g3 = ph.tile([128, 512], f32, tag="pwh")
                nc.tensor.matmul(p_g3, W["Wg3"], gp[:, sl], start=True, stop=True)
                nc.scalar.activation(th[:, sl], p_g3, AF.Tanh, scale=0.5, bias=tanh_bias)
                nc.vector.scalar_tensor_tensor(
                    dlt[:, sl], th[:, sl], 1.0, u[:, sl], op0=OP.add, op1=OP.mult
                )
                nc.sync.dma_start_transpose(dlt_nat[:, cs, :], dlt[:, sl])
                nc.gpsimd.tensor_add(o[:, cs, :], dlt_nat[:, cs, :], x_nat[:, cs, :])
                nc.sync.dma_start(out_r[:, cs, :], o[:, cs, :])

        # Software pipeline: emit P23(k-2), P1(k-1), Load(k) per step so each
        # engine's in-order stream interleaves two samples and input DMAs run
        # a full step ahead of first use.
        for k in range(1, BPC + 2):
            if 0 <= k - 1 < BPC:
                phase1(ST[k - 1])
            if 0 <= k - 2:
                phase23(k - 2, ST[k - 2])
            if k < BPC:
                ST[k] = load(k)

    # Force Exp and Tanh to resolve to the one table set that holds both
    # (exp_and_others): contents-only lie to the set chooser, dict order
    # (= act_func_set_id) preserved; the set actually loaded at runtime does
    # contain both functions (plus Identity/Relu used by bias fallbacks).
    import concourse.bacc as bacc_mod

    real_get = bacc_mod.get_activation_tables
    target = "exp_and_others"

    def patched_get(arch):
        tabs = real_get(arch)
        strip = {AF.Exp, AF.Tanh}
        return {
            name: (set(fns) if name == target else set(fns) - strip)
            for name, fns in tabs.items()
        }

    bacc_mod.get_activation_tables = patched_get
    try:
        nc.compile()
    finally:
        bacc_mod.get_activation_tables = real_get
    return nc


def _prep_host(inputs):
    """Host-side: fold weights/biases; returns (f32 inputs, wpack bf16, biases)."""
    f32 = np.float32
    g = {k: np.asarray(v, f32) for k, v in inputs.items()}

    Wm = g["Wk"] @ g["Wq"].T                       # y = x@Wk@Wq^T; logit=x_i.y_j
    Wvo = g["Wv"] @ g["Wo"]                        # v path folded into tail
    Wo1m = g["Wo1"] - np.eye(D, dtype=f32)
    Wog2 = Wvo @ g["Wg2"]                          # msg path folded into gate
    bo_msg = g["bo"] + g["bv"] @ g["Wo"]           # bv folded through Wo
    bo_uh = 0.5 * (bo_msg + g["bo1"])              # msg bias + ret bias, halved
    bo_g = bo_msg @ g["Wg2"] + g["bg1"] + g["bg2"]
    bg3h = 0.5 * g["bg3"]                          # tanh((z+bg3)/2) bias
    by = g["Wk"] @ g["bq"]                         # per-key logit bias

    wmap = {
        "Wm": Wm, "Woh": 0.5 * Wvo, "Wo1mh": 0.5 * Wo1m,
        "Wg1": g["Wg1"], "Wog2": Wog2, "Wg3": g["Wg3"],
    }
    bmap = {
        "by": by,
        "bo_uh": bo_uh, "bo_g": bo_g, "bg3h": bg3h,
    }
    bf16 = ml_dtypes.bfloat16
    wpack = np.stack([wmap[n] for n in WNAMES], axis=1).astype(bf16)
    return g, np.ascontiguousarray(wpack), bmap


def _prep_inputs(inputs):
    g, wpack, bmap = _prep_host(inputs)
    modes = {n: _bias_mode(v) for n, v in bmap.items()}
    base = {"wpack": wpack}
    for n, v in bmap.items():
        if modes[n][0] == "ap":
            base[n] = np.ascontiguousarray(v.reshape(D, 1).astype(np.float32))
    xbf = np.ascontiguousarray(g["x"].astype(ml_dtypes.bfloat16))
    xf8 = np.ascontiguousarray(g["x"].astype(ml_dtypes.float8_e4m3fn))
    in_maps = []
    for c in range(NCORES):
        m = dict(base)
        m["xbf"] = np.ascontiguousarray(xbf[c * BPC:(c + 1) * BPC])
        m["xf8"] = np.ascontiguousarray(xf8[c * BPC:(c + 1) * BPC])
        in_maps.append(m)
    return in_maps, modes


def kernel(**inputs):
    from concourse.bass_utils import run_bass_kernel_spmd

    in_maps, modes = _prep_inputs(inputs)
    key = tuple(sorted((n, k[0], k[1]) for n, k in modes.items()))
    if _CACHE.get("key") != key:
        _CACHE["nc"] = _build_nc(modes)
        _CACHE["key"] = key
    nc = _CACHE["nc"]

    res = run_bass_kernel_spmd(nc, in_maps, list(range(NCORES)))
    out = np.concatenate([r["out"] for r in res.results], axis=0)
    return out.astype(np.float32)


# revision 10
# speedup vs baseline: 1.1826x; 1.0117x over previous
"""Trainium2 Bass kernel for nn_Net_5488968204310 (gnn_message_passing).

Single-head self-attention (D=128) over N=1024 nodes + gated residual update,
batch B=32, data-parallel across 8 NeuronCores (4 samples per core).

Design notes:
  - "T layout" (features d on partitions, nodes on free dim) for every matmul;
    contraction is always over d.
  - QK fold: logit_ij = x_i.(M^T x_j) with M = Wq@Wk^T folded on the host, so
    only ONE projection matmul (yT = (Wk Wq^T)^T... stationary S=Wk@Wq^T) is
    needed and the logits' moving operand is xT itself. The per-query bias
    term is softmax-invariant and dropped; the per-key term folds into y's
    bias (Wk@bq).
  - V fold: attn@Wo = (A@X)@(Wv@Wo). Attention runs over raw X (host-cast
    fp8 upload, natural layout) and Wv folds into the tail weights, removing
    the v projection and its PSUM->SBUF cast.
  - QK^T: logitsT chunks [m_chunk(128) x q(1024)] = yT_chunk.T @ xT; exp() on
    the scalar engine straight from PSUM with the 1/sqrt(D) scale and a -2
    bias folded in (rescales exp into fp8 range; the uniform e^-2 factor
    cancels between numerator and denominator).
  - A@X keeps xf8 as the stationary operand (DoubleRow fp8, dense 512-col
    streams). The softmax denominator is a parallel ones.T @ expw DoubleRow
    accumulation; 1/denom via the DVE reciprocal_approx_fast custom op.
  - gate sigmoid as 0.5*(1+tanh(z/2)): one ACT op (Tanh) instead of a 3-op
    exp/ln chain. The 0.5 gate factor is folded into the u-path weights on
    the host so dlt = u_half * (tanh+1) is a single DVE scalar_tensor_tensor.
  - ACT functions used (Exp, Tanh, Identity/Relu fallbacks) all live in the
    exp_and_others table set, enforced by a scoped patch of the table
    metadata at compile time: exactly one ACT_TABLE_LOAD per run.
  - The residual add runs in bf16 on the Pool engine (x loaded bf16); all
    six folded 128x128 weights ship as ONE packed DMA to cut issue latency.
"""

import math

import numpy as np
import ml_dtypes

B, N, D = 32, 1024, 128
NCORES = 8
BPC = B // NCORES  # samples per core
NT = N // 128      # node chunks per sample

WNAMES = ["Wm", "Woh", "Wo1mh", "Wg1", "Wog2", "Wg3"]

_CACHE = {}


def _bias_mode(vec):
    """(kind, value) where kind in {'zero', 'uniform', 'ap'}."""
    v = np.asarray(vec, np.float32)
    if not np.any(v):
        return ("zero", 0.0)
    if np.all(v == v.flat[0]):
        return ("uniform", float(v.flat[0]))
    return ("ap", 0.0)


def _build_nc(modes):
    import concourse.bacc as bacc
    import concourse.tile as tile
    from concourse import mybir
    from contextlib import ExitStack

    f32 = mybir.dt.float32
    bf16 = mybir.dt.bfloat16
    f8 = mybir.dt.float8e4
    AF = mybir.ActivationFunctionType
    OP = mybir.AluOpType

    nc = bacc.Bacc("TRN2", target_bir_lowering=False, debug=False)

    xb_d = nc.dram_tensor("xbf", [BPC, N, D], bf16, kind="ExternalInput")
    xt_d = nc.dram_tensor("xtb", [BPC, D, N], bf16, kind="ExternalInput")
    xf8_d = nc.dram_tensor("xf8", [BPC, N, D], f8, kind="ExternalInput")
    out_d = nc.dram_tensor("out", [BPC, N, D], f32, kind="ExternalOutput")
    wp_d = nc.dram_tensor("wpack", [D, len(WNAMES), D], bf16, kind="ExternalInput")
    b_d = {
        n: nc.dram_tensor(n, [D, 1], f32, kind="ExternalInput")
        for n in modes if modes[n][0] == "ap"
    }

    s = 1.0 / math.sqrt(D)

    with tile.TileContext(nc) as tc, ExitStack() as ctx:
        consts = ctx.enter_context(tc.tile_pool(name="consts", bufs=1))
        sb = ctx.enter_context(tc.tile_pool(name="sb", bufs=2))
        sb3 = ctx.enter_context(tc.tile_pool(name="sb3", bufs=3))
        expp = ctx.enter_context(tc.tile_pool(name="expp", bufs=2))
        pw = ctx.enter_context(tc.tile_pool(name="pw", bufs=2, space="PSUM"))
        ph = ctx.enter_context(tc.tile_pool(name="ph", bufs=2, space="PSUM"))
        pav = ctx.enter_context(tc.tile_pool(name="pav", bufs=1, space="PSUM"))
        pden = ctx.enter_context(tc.tile_pool(name="pden", bufs=1, space="PSUM"))

        ST = {}

        def load(b):
            """input DMAs for sample b (issued one pipeline step early)."""
            st = {}
            xT = sb3.tile([128, N], bf16, tag="xT")  # [d, n]
            nc.sync.dma_start(xT, xt_d[b])
            x_nat = sb3.tile([128, NT, D], bf16, tag="x_nat")
            nc.sync.dma_start(x_nat, xb_d[b].rearrange("(c p) d -> p c d", p=128))
            xf8_nat = sb3.tile([128, NT, D], f8, tag="xf8_nat")
            nc.sync.dma_start(xf8_nat, xf8_d[b].rearrange("(c p) d -> p c d", p=128))
            st["x_nat"], st["xf8_nat"], st["xT"] = x_nat, xf8_nat, xT
            return st

        # input DMAs for sample 0 go out before anything else; the packed
        # weight DMA + consts follow on other engines so nothing serializes
        # behind the sequencer's per-DMA issue cost.
        ST[0] = load(0)

        wpack = consts.tile([D, len(WNAMES), D], bf16, tag="wpack")
        nc.scalar.dma_start(wpack, wp_d[:, :, :])
        W = {n: wpack[:, i, :] for i, n in enumerate(WNAMES)}
        ones_dr = consts.tile([128, 2, 128], f8, tag="ones_dr")
        nc.gpsimd.memset(ones_dr, 1.0)
        expbias = consts.tile([128, 1], f32, tag="expbias")
        nc.gpsimd.memset(expbias, -2.0)
        BV = {}
        for n in b_d:
            t = consts.tile([D, 1], f32, tag=f"b_{n}")
            nc.scalar.dma_start(t, b_d[n][:, :])
            BV[n] = t
        for n, (kind, val) in modes.items():
            if kind == "uniform":
                t = consts.tile([D, 1], f32, tag=f"b_{n}")
                nc.gpsimd.memset(t, val)
                BV[n] = t

        def copyback(dst, src, bname, engine_copy):
            """psum->sbuf copy honoring the bias mode for `bname`."""
            kind, val = modes[bname]
            if kind == "zero":
                engine_copy(dst, src)
            else:
                nc.scalar.activation(dst, src, AF.Identity, bias=BV[bname])

        def phase1(st):
            """y projection, QK^T + exp."""
            xT = st["xT"]

            p_y = pw.tile([128, N], f32, tag="pw")
            nc.tensor.matmul(p_y[:, 0:512], W["Wm"], xT[:, 0:512], start=True, stop=True)
            nc.tensor.matmul(p_y[:, 512:1024], W["Wm"], xT[:, 512:1024], start=True, stop=True)
            yT = sb.tile([128, N], bf16, tag="yT")
            copyback(yT, p_y, "by", nc.vector.tensor_copy)

            expw = expp.tile([128, NT, N], f8, tag="expw")  # [m', c_m, q]
            for c in range(NT):
                p_l = pw.tile([128, N], f32, tag="pw")
                yTc = yT[:, c * 128:(c + 1) * 128]
                nc.tensor.matmul(p_l[:, 0:512], yTc, xT[:, 0:512], start=True, stop=True)
                nc.tensor.matmul(p_l[:, 512:1024], yTc, xT[:, 512:1024], start=True, stop=True)
                nc.scalar.activation(expw[:, c, :], p_l, AF.Exp, scale=s, bias=expbias)
            st["expw"] = expw

        def phase23(b, st):
            """softmax normalize + gated update tail; store.

            u = 0.5*(ret - x) comes straight out of the folded-weight matmul;
            gate = 0.5*(1 + tanh((z+bg3)/2)); out = x + u*(1+tanh)."""
            expw, xf8_nat = st["expw"], st["xf8_nat"]
            x_nat, xT = st["x_nat"], st["xT"]
            rb = sb.tile([128, N], f32, tag="rb")
            attnT = sb.tile([128, N], bf16, tag="attnT")
            u = sb.tile([128, N], f32, tag="u")
            gp = sb.tile([128, N], bf16, tag="gp")
            th = sb.tile([128, N], bf16, tag="th")
            dlt = sb.tile([128, N], bf16, tag="dlt")
            dlt_nat = sb.tile([128, NT, 128], bf16, tag="dlt_nat")
            o = sb.tile([128, NT, D], f32, tag="o")
            out_r = out_d[b].rearrange("(c p) d -> p c d", p=128)
            H = NT // 2
            tanh_bias = BV["bg3h"] if "bg3h" in BV else 0.0
            for h in range(2):
                sl = slice(h * 512, (h + 1) * 512)
                p_dn = pden.tile([128, 512], f32, tag="pden")
                for c in range(NT // 2):
                    nc.tensor.matmul(
                        p_dn, ones_dr, expw[:, 2 * c:2 * c + 2, sl],
                        start=(c == 0), stop=(c == NT // 2 - 1),
                        perf_mode=mybir.MatmulPerfMode.DoubleRow,
                    )
                nc.vector.reciprocal_approx_fast(rb[:, sl], p_dn)
                p_av = pav.tile([128, 512], f32, tag="pav")
                for c in range(NT // 2):
                    nc.tensor.matmul(
                        p_av, xf8_nat[:, 2 * c:2 * c + 2, :], expw[:, 2 * c:2 * c + 2, sl],
                        start=(c == 0), stop=(c == NT // 2 - 1),
                        perf_mode=mybir.MatmulPerfMode.DoubleRow,
                    )
                nc.vector.tensor_mul(attnT[:, sl], p_av, rb[:, sl])
            for h in range(2):
                sl = slice(h * 512, (h + 1) * 512)
                cs = slice(h * H, (h + 1) * H)

                p_m = ph.tile([128, 512], f32, tag="pwh")
                nc.tensor.matmul(p_m, W["Woh"], attnT[:, sl], start=True, stop=False)
                nc.tensor.matmul(p_m, W["Wo1mh"], xT[:, sl], start=False, stop=True)
                copyback(u[:, sl], p_m, "bo_uh", nc.vector.tensor_copy)

                p_g = ph.tile([128, 512], f32, tag="pwh")
                nc.tensor.matmul(p_g, W["Wg1"], xT[:, sl], start=True, stop=False)
                nc.tensor.matmul(p_g, W["Wog2"], attnT[:, sl], start=False, stop=True)
                if modes["bo_g"][0] == "zero":
                    nc.vector.tensor_scalar(gp[:, sl], p_g, 0.0, None, op0=OP.max)
                else:
                    nc.scalar.activation(gp[:, sl], p_g, AF.Relu, bias=BV["bo_g"])

                p_g3 = ph.tile([128, 512], f32, tag="pwh")
                nc.tensor.matmul(p_g3, W["Wg3"], gp[:, sl], start=True, stop=True)
                nc.scalar.activation(th[:, sl], p_g3, AF.Tanh, scale=0.5, bias=tanh_bias)
                nc.vector.scalar_tensor_tensor(
                    dlt[:, sl], th[:, sl], 1.0, u[:, sl], op0=OP.add, op1=OP.mult
                )
                last = b == BPC - 1
                teng = nc.scalar if (last and h == 1) else nc.sync
                teng.dma_start_transpose(dlt_nat[:, cs, :], dlt[:, sl])
                if last:
                    nc.vector.tensor_add(o[:, cs, :], dlt_nat[:, cs, :], x_nat[:, cs, :])
                    nc.sync.dma_start(out_r[:, cs, :], o[:, cs, :])
                else:
                    nc.gpsimd.tensor_add(o[:, cs, :], dlt_nat[:, cs, :], x_nat[:, cs, :])
                    nc.gpsimd.dma_start(out_r[:, cs, :], o[:, cs, :])

        # Software pipeline: emit P23(k-2), P1(k-1), Load(k) per step so each
        # engine's in-order stream interleaves two samples and input DMAs run
        # a full step ahead of first use.
        for k in range(1, BPC + 2):
            if 0 <= k - 1 < BPC:
                phase1(ST[k - 1])
            if 0 <= k - 2:
                phase23(k - 2, ST[k - 2])
            if k < BPC:
                ST[k] = load(k)

    # Force Exp and Tanh to resolve to the one table set that holds both
    # (exp_and_others): contents-only lie to the set chooser, dict order
    # (= act_func_set_id) preserved; the set actually loaded at runtime does
    # contain both functions (plus Identity/Relu used by bias fallbacks).
    import concourse.bacc as bacc_mod

    real_get = bacc_mod.get_activation_tables
    target = "exp_and_others"

    def patched_get(arch):
        tabs = real_get(arch)
        strip = {AF.Exp, AF.Tanh}
        return {
            name: (set(fns) if name == target else set(fns) - strip)
            for name, fns in tabs.items()
        }

    bacc_mod.get_activation_tables = patched_get
    try:
        nc.compile()
    finally:
        bacc_mod.get_activation_tables = real_get
    return nc


def _prep_host(inputs):
    """Host-side: fold weights/biases; returns (f32 inputs, wpack bf16, biases)."""
    f32 = np.float32
    g = {k: np.asarray(v, f32) for k, v in inputs.items()}

    Wm = g["Wk"] @ g["Wq"].T                       # y = x@Wk@Wq^T; logit=x_i.y_j
    Wvo = g["Wv"] @ g["Wo"]                        # v path folded into tail
    Wo1m = g["Wo1"] - np.eye(D, dtype=f32)
    Wog2 = Wvo @ g["Wg2"]                          # msg path folded into gate
    bo_msg = g["bo"] + g["bv"] @ g["Wo"]           # bv folded through Wo
    bo_uh = 0.5 * (bo_msg + g["bo1"])              # msg bias + ret bias, halved
    bo_g = bo_msg @ g["Wg2"] + g["bg1"] + g["bg2"]
    bg3h = 0.5 * g["bg3"]                          # tanh((z+bg3)/2) bias
    by = g["Wk"] @ g["bq"]                         # per-key logit bias

    wmap = {
        "Wm": Wm, "Woh": 0.5 * Wvo, "Wo1mh": 0.5 * Wo1m,
        "Wg1": g["Wg1"], "Wog2": Wog2, "Wg3": g["Wg3"],
    }
    bmap = {
        "by": by,
        "bo_uh": bo_uh, "bo_g": bo_g, "bg3h": bg3h,
    }
    bf16 = ml_dtypes.bfloat16
    wpack = np.stack([wmap[n] for n in WNAMES], axis=1).astype(bf16)
    return g, np.ascontiguousarray(wpack), bmap


def _prep_inputs(inputs):
    g, wpack, bmap = _prep_host(inputs)
    modes = {n: _bias_mode(v) for n, v in bmap.items()}
    base = {"wpack": wpack}
    for n, v in bmap.items():
        if modes[n][0] == "ap":
            base[n] = np.ascontiguousarray(v.reshape(D, 1).astype(np.float32))
    xbf = np.ascontiguousarray(g["x"].astype(ml_dtypes.bfloat16))
    xtb = np.ascontiguousarray(np.swapaxes(xbf, 1, 2))
    xf8 = np.ascontiguousarray(g["x"].astype(ml_dtypes.float8_e4m3fn))
    in_maps = []
    for c in range(NCORES):
        m = dict(base)
        m["xbf"] = np.ascontiguousarray(xbf[c * BPC:(c + 1) * BPC])
        m["xtb"] = np.ascontiguousarray(xtb[c * BPC:(c + 1) * BPC])
        m["xf8"] = np.ascontiguousarray(xf8[c * BPC:(c + 1) * BPC])
        in_maps.append(m)
    return in_maps, modes


def kernel(**inputs):
    from concourse.bass_utils import run_bass_kernel_spmd

    in_maps, modes = _prep_inputs(inputs)
    key = tuple(sorted((n, k[0], k[1]) for n, k in modes.items()))
    if _CACHE.get("key") != key:
        _CACHE["nc"] = _build_nc(modes)
        _CACHE["key"] = key
    nc = _CACHE["nc"]

    res = run_bass_kernel_spmd(nc, in_maps, list(range(NCORES)))
    out = np.concatenate([r["out"] for r in res.results], axis=0)
    return out.astype(np.float32)
